# revision 6
# baseline (speedup 1.0000x reference)
"""Trainium2 Bass kernel for the GCNEncoder problem (v3.2).

Strategy (window dataflow, minimal PE + ALU work):
  - Pure data parallelism: batch 65536 = 8192 per core x 8 cores.
  - Stage A (h1): 9 windows, fp8e4 DoubleRow matmuls with a 2-term (value +
    residual) fp8 expansion of both x and C1 = kron(A,W1): full 4-term
    product in one DR matmul per window (256 PE cycles instead of 512).
  - Aggregation t = (unnormalized) neighbor-sum of h1 done with 8 bf16
    tensor_tensor adds on DVE/GpSimd; window pairings chosen (exact search)
    so each t-window is a sum of full 128-partition h1 windows; joint 7's
    hi-copy is synthesized by an idle SBUF->SBUF DMA.
  - Stage B (h2): one bf16 matmul per window, lhsT = blockdiag(W2/deg_a,
    W2/deg_b) (deg-normalization folded into the weights).
  - Stage C (z): 9 accumulating bf16 matmuls vs D = Wp1 @ kron(A,W3).
  - Stage D: K=65 (bias row), 4 matmuls N=256 into one (128,1024) psum.
  - GPSIMD cannot touch PSUM, so all PSUM evacuations run on ScalarE/
    VectorE as (128,1024) double-window ops where possible (shared 2-bank
    psum tiles), balanced by a greedy build-time scheduler.
"""

import os
import sys

for _p in ("/opt/trn_rl_repo", "/root/.axon_site/_ro/trn_rl_repo"):
    if os.path.isdir(_p) and _p not in sys.path:
        sys.path.insert(0, _p)

import numpy as np
import ml_dtypes

from concourse import bacc, mybir, tile

NJ = 17
DIN = 3
H = 64
DOUT = 256
NCORES = 8
B_TOTAL = 65536
BC = B_TOTAL // NCORES          # 8192
TILE_N = 512
NTILES = BC // TILE_N           # 16
KA = 102                        # stage-A DR partition count (2*51)

F32 = mybir.dt.float32
BF16 = mybir.dt.bfloat16
F8 = mybir.dt.float8e4
NP_F8 = ml_dtypes.float8_e4m3
NP_BF = ml_dtypes.bfloat16

CONNS = [(0, 7), (7, 8), (8, 9), (9, 10), (0, 1), (1, 2), (2, 3), (0, 4),
         (4, 5), (5, 6), (8, 11), (11, 12), (12, 13), (8, 14), (14, 15), (15, 16)]

# h1 window w = (lo joint, hi joint); from exact pairing search.
H1WIN = [(0, 8), (1, 11), (2, 12), (3, 13), (4, 14), (5, 15), (6, 16), (7, 9),
         (8, 10)]
# h1 slot order in h1_sb (so evac pairs are column-adjacent); t0 chain's
# inputs (w1, w4, w7) come first.
A_ORDER = [1, 4, 7, 0, 2, 3, 5, 6, 8]
SLOT_OF = {w: s for s, w in enumerate(A_ORDER)}

# t/h2 window q = (lo joint, hi joint); q8 holds joint 10 at lo only.
H2WIN = [(0, 8), (1, 11), (2, 12), (3, 13), (4, 14), (5, 15), (6, 16), (7, 9),
         (10, None)]
# t-window construction: q -> ("alias", h1win) / ("alias_hi", h1win) /
# ("sum", [h1 windows], extra); extra "h7hi" = DMA-synthesized h1[7]@hi.
TPLAN = {
    0: ("sum", [1, 4, 7], "h7hi"),
    1: ("sum", [0, 2], None),
    2: ("sum", [1, 3], None),
    3: ("alias", 2),
    4: ("sum", [0, 5], None),
    5: ("sum", [4, 6], None),
    6: ("alias", 5),
    7: ("sum", [0, 8], None),
    8: ("alias_hi", 7),          # rhs = h1 win 7 hi half = h1[9]; N(10)={9}
}
# t slots in t_sb for the "sum" windows
TSLOT = {0: 0, 1: 1, 2: 2, 4: 3, 5: 4, 7: 5}

LAST_RESULTS = None


def _neighbors():
    nbr = [set() for _ in range(NJ)]
    for a, b in CONNS:
        nbr[a].add(b)
        nbr[b].add(a)
    return nbr


def _f8(a):
    return np.asarray(a, np.float32).astype(NP_F8)


def _build_constants(A, W1, b1, W2, b2, W3, b3, Wp1, bp1, Wp2, bp2):
    A = np.asarray(A, np.float32)
    W1 = np.asarray(W1, np.float32)
    W2 = np.asarray(W2, np.float32)
    W3 = np.asarray(W3, np.float32)
    Wp1 = np.asarray(Wp1, np.float32)
    Wp2 = np.asarray(Wp2, np.float32)
    b1 = np.asarray(b1, np.float32)
    b2 = np.asarray(b2, np.float32)
    b3 = np.asarray(b3, np.float32)
    bp1 = np.asarray(bp1, np.float32)
    bp2 = np.asarray(bp2, np.float32)
    nbr = _neighbors()
    deg = np.array([len(nbr[j]) for j in range(NJ)], np.float32)

    C1 = np.kron(A, W1)                    # (1088, 51)
    C3 = np.kron(A, W3)
    D = Wp1 @ C3                           # (64, 1088)
    bzt = Wp1 @ np.tile(b3, NJ) + bp1      # (64,)

    C1a = _f8(C1).astype(np.float32)
    C1r = _f8(C1 - C1a).astype(np.float32)

    # g1: (102, 2, 9*128) fp8 DR lhsT (indexed by h1 SLOT, not window id).
    #   both planes rows 0-50 = C1a[rows_w].T, rows 51-101 = C1r[rows_w].T;
    #   paired with x planes (xa; xa) and (xr; xr): full 4-term product.
    g1 = np.zeros((KA, 2, 9 * 128), np.float32)
    for w, (u, v) in enumerate(H1WIN):
        s = SLOT_OF[w]
        rows = np.concatenate([np.arange(u * H, (u + 1) * H),
                               np.arange(v * H, (v + 1) * H)])
        blk_a = C1a[rows].T                # (51, 128)
        blk_r = C1r[rows].T
        for pl in range(2):
            g1[0:51, pl, 128 * s:128 * (s + 1)] = blk_a
            g1[51:102, pl, 128 * s:128 * (s + 1)] = blk_r

    # g2: (128, 9*128) bf16 stage-B lhsT; out = lhsT.T @ t.
    # window q (a,b): lhsT[k, m<64] = W2[m, k]/deg_a;
    #                 lhsT[64+k, 64+m] = W2[m, k]/deg_b.
    # q8 (single joint 10): block at partitions 64-127 (rhs lives at hi),
    # M-cols 64-127 zero (zeroes psum hi half on start=True).
    g2 = np.zeros((128, 9 * 128), np.float32)
    for q, (a, b) in enumerate(H2WIN):
        if b is not None:
            g2[0:64, 128 * q:128 * q + 64] = W2.T / deg[a]
            g2[64:128, 128 * q + 64:128 * q + 128] = W2.T / deg[b]
        else:
            g2[64:128, 128 * q:128 * q + 64] = W2.T / deg[a]

    # g3: (128, 9*64) bf16 stage-C lhsT: lhsT[k, m] = D[m, a*64+k],
    # lhsT[64+k, m] = D[m, b*64+k]; q8 hi rows zero.
    g3 = np.zeros((128, 9 * 64), np.float32)
    for q, (a, b) in enumerate(H2WIN):
        g3[0:64, 64 * q:64 * (q + 1)] = D[:, a * H:(a + 1) * H].T
        if b is not None:
            g3[64:128, 64 * q:64 * (q + 1)] = D[:, b * H:(b + 1) * H].T

    wp2t = np.zeros((65, DOUT), np.float32)
    wp2t[0:64] = Wp2.T
    wp2t[64] = bp2

    return {
        "g1": g1.astype(NP_F8),
        "g2": g2.astype(NP_BF),
        "g3": g3.astype(NP_BF),
        "wp2t": wp2t.astype(NP_BF),
        "b1p": np.tile(b1, 2).reshape(128, 1).copy(),
        "b2p": np.tile(b2, 2).reshape(128, 1).copy(),
        "bzt": bzt.reshape(64, 1).copy(),
    }


def _pack_x(x_core):
    """x_core (n, 51) fp32 -> (102, 2, n) fp8 DR moving operand."""
    xT = np.ascontiguousarray(x_core.T)          # (51, n)
    xa = _f8(xT)
    xr = _f8(xT - xa.astype(np.float32))
    x2 = np.zeros((KA, 2, xT.shape[1]), NP_F8)
    x2[0:51, 0] = xa
    x2[51:102, 0] = xa
    x2[0:51, 1] = xr
    x2[51:102, 1] = xr
    return x2


class _Sched:
    """Greedy build-time load balancer.

    PSUM evacuations can only run on scalar (act) / vector (dve).
    SBUF-only adds can also run on gpsimd (pool).
    """
    EVAC_COST = {"act": {512: 611.0, 1024: 1038.0},
                 "dve": {512: 658.0, 1024: 1192.0}}
    AGG_COST = {"dve": 330.0, "pool": 1150.0}

    def __init__(self, nc):
        self.nc = nc
        self.load = {"act": 0.0, "dve": 0.0, "pool": 0.0}
        self.n = {"act": 0, "dve": 0, "pool": 0}

    def _pick(self, table):
        e = min(table, key=lambda k: self.load[k] + table[k])
        self.load[e] += table[e]
        self.n[e] += 1
        return e

    def evac(self, dst, src, bias, width):
        nc = self.nc
        e = self._pick({k: v[width] for k, v in self.EVAC_COST.items()})
        ALU = mybir.AluOpType
        if e == "act":
            nc.scalar.activation(dst, src, mybir.ActivationFunctionType.Relu,
                                 bias=bias)
        else:
            nc.vector.tensor_scalar(out=dst, in0=src, scalar1=bias,
                                    scalar2=0.0, op0=ALU.add, op1=ALU.max)

    def copy(self, dst, src, width):
        nc = self.nc
        e = self._pick({k: v[width] for k, v in self.EVAC_COST.items()})
        if e == "act":
            nc.scalar.copy(dst, src)
        else:
            nc.vector.tensor_copy(dst, src)

    def agg(self, out, in0, in1):
        nc = self.nc
        e = self._pick(self.AGG_COST)
        if e == "dve":
            nc.vector.tensor_tensor(out=out, in0=in0, in1=in1,
                                    op=mybir.AluOpType.add)
        else:
            nc.gpsimd.tensor_tensor(out=out, in0=in0, in1=in1,
                                    op=mybir.AluOpType.add)


def _build_program(reps=1):
    nc = bacc.Bacc(None)

    x2_d = nc.declare_dram_parameter("x2", [KA, 2, BC], F8, isOutput=False)
    g1_d = nc.declare_dram_parameter("g1", [KA, 2, 9 * 128], F8, isOutput=False)
    g2_d = nc.declare_dram_parameter("g2", [128, 9 * 128], BF16, isOutput=False)
    g3_d = nc.declare_dram_parameter("g3", [128, 9 * 64], BF16, isOutput=False)
    wp2t_d = nc.declare_dram_parameter("wp2t", [65, DOUT], BF16, isOutput=False)
    b1p_d = nc.declare_dram_parameter("b1p", [128, 1], F32, isOutput=False)
    b2p_d = nc.declare_dram_parameter("b2p", [128, 1], F32, isOutput=False)
    bzt_d = nc.declare_dram_parameter("bzt", [H, 1], F32, isOutput=False)
    out_d = nc.declare_dram_parameter("out", [BC, DOUT], BF16, isOutput=True)

    out_r = out_d.rearrange("(c p) f -> p c f", p=128)      # (128, 64, 256)

    MM = mybir.MatmulPerfMode

    with tile.TileContext(nc) as tc:
        with (
            tc.tile_pool(name="const", bufs=1) as cp,
            tc.tile_pool(name="h1", bufs=2) as h1p,
            tc.tile_pool(name="tt", bufs=2) as tp,
            tc.tile_pool(name="h7", bufs=2) as h7p,
            tc.tile_pool(name="h2", bufs=2) as h2p,
            tc.tile_pool(name="ot", bufs=3) as otp,
            tc.tile_pool(name="psm", bufs=3, space="PSUM") as psm,
            tc.tile_pool(name="psc", bufs=2, space="PSUM") as psc,
        ):
            x2_sb = cp.tile([KA, 2, BC], F8)
            g1_sb = cp.tile([KA, 2, 9 * 128], F8)
            g2_sb = cp.tile([128, 9 * 128], BF16)
            g3_sb = cp.tile([128, 9 * 64], BF16)
            wp2t_sb = cp.tile([65, DOUT], BF16)
            b1p_sb = cp.tile([128, 1], F32)
            b2p_sb = cp.tile([128, 1], F32)
            bzt_sb = cp.tile([H, 1], F32)
            z4_sb = cp.tile([65, 2 * TILE_N], BF16)   # manual double buffer

            nc.sync.dma_start(x2_sb[:, :, :], x2_d[:, :, :])
            nc.sync.dma_start(g1_sb[:, :, :], g1_d[:, :, :])
            nc.sync.dma_start(g2_sb[:], g2_d[:])
            nc.sync.dma_start(g3_sb[:], g3_d[:])
            nc.sync.dma_start(wp2t_sb[:], wp2t_d[:])
            nc.sync.dma_start(b1p_sb[:], b1p_d[:])
            nc.sync.dma_start(b2p_sb[:], b2p_d[:])
            nc.sync.dma_start(bzt_sb[:], bzt_d[:])
            nc.gpsimd.memset(z4_sb[64:65, :], 1.0)    # stage-D bias row

            sched = _Sched(nc)
            state = {}

            def a_pair(g, h1_sb, s0, nwin):
                """A matmuls for slots s0..s0+nwin-1 into one psum tile,
                one evac."""
                ps = psm.tile([128, 2 * TILE_N], F32, name="ps_m")
                for i in range(nwin):
                    s = s0 + i
                    nc.tensor.matmul(
                        ps[:, TILE_N * i:TILE_N * (i + 1)],
                        g1_sb[:, :, 128 * s:128 * (s + 1)],
                        x2_sb[:, :, TILE_N * g:TILE_N * (g + 1)],
                        start=True, stop=True,
                        perf_mode=MM.DoubleRow,
                    )
                w = nwin * TILE_N
                sched.evac(h1_sb[:, TILE_N * s0:TILE_N * s0 + w], ps[:, 0:w],
                           b1p_sb[:], w)

            def stage_a_pre(g):
                h1_sb = h1p.tile([128, 9 * TILE_N], BF16)
                h7_sb = h7p.tile([128, TILE_N], BF16)
                state[("h1", g)] = h1_sb
                state[("h7", g)] = h7_sb
                a_pair(g, h1_sb, 0, 2)        # w1, w4

            def stage_a_mid(g):
                h1_sb = state[("h1", g)]
                h7_sb = state[("h7", g)]
                a_pair(g, h1_sb, 2, 2)        # w7, w0
                # synthesize h1[7] at hi partitions via DMA (slot 2 = w7)
                nc.sync.dma_start(
                    h7_sb[64:128, :],
                    h1_sb[0:64, TILE_N * 2:TILE_N * 3],
                )

            def stage_a_post(g):
                h1_sb = state[("h1", g)]
                h7_sb = state[("h7", g)]
                a_pair(g, h1_sb, 4, 2)        # w2, w3
                a_pair(g, h1_sb, 6, 2)        # w5, w6
                a_pair(g, h1_sb, 8, 1)        # w8
                # aggregation: t windows (bf16 adds, DVE/GpSimd)
                t_sb = tp.tile([128, 6 * TILE_N], BF16)
                state[("t", g)] = t_sb

                def hw(w):
                    s = SLOT_OF[w]
                    return h1_sb[:, TILE_N * s:TILE_N * (s + 1)]

                def ts(q):
                    s = TSLOT[q]
                    return t_sb[:, TILE_N * s:TILE_N * (s + 1)]

                for q in [1, 2, 4, 5, 7]:
                    ws = TPLAN[q][1]
                    sched.agg(ts(q), hw(ws[0]), hw(ws[1]))
                # q0: 3-term chain + h7hi half (chain stays on dve)
                nc.vector.tensor_tensor(out=ts(0), in0=hw(1), in1=hw(4),
                                        op=mybir.AluOpType.add)
                nc.vector.tensor_tensor(out=ts(0), in0=ts(0), in1=hw(7),
                                        op=mybir.AluOpType.add)
                nc.vector.tensor_tensor(out=ts(0)[64:128, :],
                                        in0=ts(0)[64:128, :],
                                        in1=h7_sb[64:128, :],
                                        op=mybir.AluOpType.add)

            def b_rhs(q, h1_sb, t_sb):
                kind = TPLAN[q][0]
                if kind == "alias":
                    return h1_sb[:, TILE_N * SLOT_OF[TPLAN[q][1]]:
                                 TILE_N * (SLOT_OF[TPLAN[q][1]] + 1)]
                if kind == "alias_hi":
                    return h1_sb[64:128, TILE_N * SLOT_OF[TPLAN[q][1]]:
                                 TILE_N * (SLOT_OF[TPLAN[q][1]] + 1)]
                s = TSLOT[q]
                return t_sb[:, TILE_N * s:TILE_N * (s + 1)]

            def b_mm(q, rhs, ps, off):
                if TPLAN[q][0] == "alias_hi":
                    nc.tensor.matmul(
                        ps[:, off:off + TILE_N],
                        g2_sb[64:128, 128 * q:128 * (q + 1)],
                        rhs, start=True, stop=True,
                        tile_position=(64, 0),
                    )
                else:
                    nc.tensor.matmul(
                        ps[:, off:off + TILE_N],
                        g2_sb[:, 128 * q:128 * (q + 1)],
                        rhs, start=True, stop=True,
                    )

            def stage_b(g):
                h1_sb = state.pop(("h1", g))
                t_sb = state.pop(("t", g))
                state.pop(("h7", g))
                h2_sb = h2p.tile([128, 9 * TILE_N], BF16)
                state[("h2", g)] = h2_sb
                for q0 in (2, 4, 6, 0):   # q0's rhs (t0) is ready last
                    ps = psm.tile([128, 2 * TILE_N], F32, name="ps_m")
                    b_mm(q0, b_rhs(q0, h1_sb, t_sb), ps, 0)
                    b_mm(q0 + 1, b_rhs(q0 + 1, h1_sb, t_sb), ps, TILE_N)
                    sched.evac(h2_sb[:, TILE_N * q0:TILE_N * (q0 + 2)],
                               ps[:, :], b2p_sb[:], 2 * TILE_N)
                ps = psm.tile([128, 2 * TILE_N], F32, name="ps_m")
                b_mm(8, b_rhs(8, h1_sb, t_sb), ps, 0)
                sched.evac(h2_sb[:, TILE_N * 8:TILE_N * 9],
                           ps[:, 0:TILE_N], b2p_sb[:], TILE_N)

            def stage_c(g):
                h2_sb = state.pop(("h2", g))
                ps_c = psc.tile([128, TILE_N], F32)
                for p in range(9):
                    nc.tensor.matmul(
                        ps_c[0:64, :],
                        g3_sb[:, 64 * p:64 * (p + 1)],
                        h2_sb[:, TILE_N * p:TILE_N * (p + 1)],
                        start=(p == 0), stop=(p == 8),
                    )
                zoff = TILE_N * (g % 2)
                sched.evac(z4_sb[0:64, zoff:zoff + TILE_N], ps_c[0:64, :],
                           bzt_sb[:], TILE_N)

            def stage_d(g):
                zoff = TILE_N * (g % 2)
                ot_sb = otp.tile([128, 4 * DOUT], BF16)
                ps_d = psm.tile([128, 2 * TILE_N], F32, name="ps_m")
                for kk in range(4):
                    nc.tensor.matmul(
                        ps_d[:, DOUT * kk:DOUT * (kk + 1)],
                        z4_sb[0:65, zoff + 128 * kk:zoff + 128 * (kk + 1)],
                        wp2t_sb[:],
                        start=True, stop=True,
                    )
                sched.copy(ot_sb[:], ps_d[:], 2 * TILE_N)
                nc.sync.dma_start(out_r[:, 4 * g:4 * (g + 1), :], ot_sb[:])

            ntiles = int(os.environ.get("KERNEL_NTILES", str(NTILES)))

            def body():
                for g in range(ntiles + 2):
                    if g < ntiles:
                        stage_a_pre(g)
                    if 1 <= g <= ntiles:
                        stage_c(g - 1)
                    if g < ntiles:
                        stage_a_mid(g)
                        stage_a_post(g)
                        stage_b(g)
                    if 2 <= g:
                        stage_d(g - 2)

            if reps == 1:
                body()
            else:
                with tc.For_i(0, reps, 1):
                    body()

    nc.compile()
    return nc


_CACHE = {}


def kernel(**inputs):
    global LAST_RESULTS
    from concourse.bass_utils import run_bass_kernel_spmd

    x = np.ascontiguousarray(np.asarray(inputs["x"], np.float32))
    consts = _build_constants(
        inputs["A"], inputs["W1"], inputs["b1"], inputs["W2"], inputs["b2"],
        inputs["W3"], inputs["b3"], inputs["Wp1"], inputs["bp1"],
        inputs["Wp2"], inputs["bp2"],
    )

    reps = int(os.environ.get("BENCH_REPS", "1"))
    key = (reps,)
    if key not in _CACHE:
        _CACHE[key] = _build_program(reps=reps)
    nc = _CACHE[key]

    xf = x.reshape(B_TOTAL, NJ * DIN)
    in_maps = []
    for c in range(NCORES):
        m = dict(consts)
        m["x2"] = _pack_x(xf[c * BC:(c + 1) * BC])
        in_maps.append(m)

    res = run_bass_kernel_spmd(nc, in_maps, list(range(NCORES)))
    LAST_RESULTS = res
    out = np.concatenate([np.asarray(res.results[c]["out"])
                          for c in range(NCORES)], axis=0)
    return out.astype(np.float32)


if __name__ == "__main__":
    import reference
    ins = {k: np.asarray(v) for k, v in reference.setup_inputs().items()}
    exp = np.asarray(reference.reference(**reference.setup_inputs()))
    os.environ.setdefault("BENCH_REPS", "1")
    act = kernel(**ins)
    scale = float(np.abs(exp).max())
    err = float(np.abs(act - exp).max())
    print(f"scale={scale:.4f} abserr={err:.3e} relerr={err / scale:.3e}")


# revision 11
# speedup vs baseline: 1.2780x; 1.2780x over previous
"""Trainium2 Bass kernel for the GCNEncoder problem (v3.2).

Strategy (window dataflow, minimal PE + ALU work):
  - Pure data parallelism: batch 65536 = 8192 per core x 8 cores.
  - Stage A (h1): 9 windows, fp8e4 DoubleRow matmuls with a 2-term (value +
    residual) fp8 expansion of both x and C1 = kron(A,W1): full 4-term
    product in one DR matmul per window (256 PE cycles instead of 512).
  - Aggregation t = (unnormalized) neighbor-sum of h1 done with 8 bf16
    tensor_tensor adds on DVE/GpSimd; window pairings chosen (exact search)
    so each t-window is a sum of full 128-partition h1 windows; joint 7's
    hi-copy is synthesized by an idle SBUF->SBUF DMA.
  - Stage B (h2): one bf16 matmul per window, lhsT = blockdiag(W2/deg_a,
    W2/deg_b) (deg-normalization folded into the weights).
  - Stage C (z): 9 accumulating bf16 matmuls vs D = Wp1 @ kron(A,W3).
  - Stage D: K=65 (bias row), 4 matmuls N=256 into one (128,1024) psum.
  - GPSIMD cannot touch PSUM, so all PSUM evacuations run on ScalarE/
    VectorE as (128,1024) double-window ops where possible (shared 2-bank
    psum tiles), balanced by a greedy build-time scheduler.
"""

import os
import sys

for _p in ("/opt/trn_rl_repo", "/root/.axon_site/_ro/trn_rl_repo"):
    if os.path.isdir(_p) and _p not in sys.path:
        sys.path.insert(0, _p)

import numpy as np
import ml_dtypes

from concourse import bacc, mybir, tile

NJ = 17
DIN = 3
H = 64
DOUT = 256
NCORES = 8
B_TOTAL = 65536
BC = B_TOTAL // NCORES          # 8192
TILE_N = 512
NTILES = BC // TILE_N           # 16
KA = 102                        # stage-A DR partition count (2*51)

F32 = mybir.dt.float32
BF16 = mybir.dt.bfloat16
F8 = mybir.dt.float8e4
NP_F8 = ml_dtypes.float8_e4m3
NP_BF = ml_dtypes.bfloat16

CONNS = [(0, 7), (7, 8), (8, 9), (9, 10), (0, 1), (1, 2), (2, 3), (0, 4),
         (4, 5), (5, 6), (8, 11), (11, 12), (12, 13), (8, 14), (14, 15), (15, 16)]

# h1 window w = (lo joint, hi joint); from exact pairing search.
H1WIN = [(0, 8), (1, 11), (2, 12), (3, 13), (4, 14), (5, 15), (6, 16), (7, 9),
         (8, 10)]
# h1 slot order in h1_sb (so evac pairs are column-adjacent); t0 chain's
# inputs (w1, w4, w7) come first.
A_ORDER = [1, 4, 7, 0, 2, 3, 5, 6, 8]
SLOT_OF = {w: s for s, w in enumerate(A_ORDER)}

# t/h2 window q = (lo joint, hi joint); q8 holds joint 10 at lo only.
H2WIN = [(0, 8), (1, 11), (2, 12), (3, 13), (4, 14), (5, 15), (6, 16), (7, 9),
         (10, None)]
# t-window construction: q -> ("alias", h1win) / ("alias_hi", h1win) /
# ("sum", [h1 windows], extra); extra "h7hi" = DMA-synthesized h1[7]@hi.
TPLAN = {
    0: ("sum", [1, 4, 7], "h7hi"),
    1: ("sum", [0, 2], None),
    2: ("sum", [1, 3], None),
    3: ("alias", 2),
    4: ("sum", [0, 5], None),
    5: ("sum", [4, 6], None),
    6: ("alias", 5),
    7: ("sum", [0, 8], None),
    8: ("alias_hi", 7),          # rhs = h1 win 7 hi half = h1[9]; N(10)={9}
}
# t slots in t_sb for the "sum" windows
TSLOT = {0: 0, 1: 1, 2: 2, 4: 3, 5: 4, 7: 5}

LAST_RESULTS = None


def _neighbors():
    nbr = [set() for _ in range(NJ)]
    for a, b in CONNS:
        nbr[a].add(b)
        nbr[b].add(a)
    return nbr


def _f8(a):
    return np.asarray(a, np.float32).astype(NP_F8)


def _build_constants(A, W1, b1, W2, b2, W3, b3, Wp1, bp1, Wp2, bp2):
    A = np.asarray(A, np.float32)
    W1 = np.asarray(W1, np.float32)
    W2 = np.asarray(W2, np.float32)
    W3 = np.asarray(W3, np.float32)
    Wp1 = np.asarray(Wp1, np.float32)
    Wp2 = np.asarray(Wp2, np.float32)
    b1 = np.asarray(b1, np.float32)
    b2 = np.asarray(b2, np.float32)
    b3 = np.asarray(b3, np.float32)
    bp1 = np.asarray(bp1, np.float32)
    bp2 = np.asarray(bp2, np.float32)
    nbr = _neighbors()
    deg = np.array([len(nbr[j]) for j in range(NJ)], np.float32)

    C1 = np.kron(A, W1)                    # (1088, 51)
    C3 = np.kron(A, W3)
    D = Wp1 @ C3                           # (64, 1088)
    bzt = Wp1 @ np.tile(b3, NJ) + bp1      # (64,)

    C1a = _f8(C1).astype(np.float32)
    C1r = _f8(C1 - C1a).astype(np.float32)

    # g1: (102, 2, 9*128) fp8 DR lhsT (indexed by h1 SLOT, not window id).
    #   both planes rows 0-50 = C1a[rows_w].T, rows 51-101 = C1r[rows_w].T;
    #   paired with x planes (xa; xa) and (xr; xr): full 4-term product.
    g1 = np.zeros((KA, 2, 9 * 128), np.float32)
    for w, (u, v) in enumerate(H1WIN):
        s = SLOT_OF[w]
        rows = np.concatenate([np.arange(u * H, (u + 1) * H),
                               np.arange(v * H, (v + 1) * H)])
        blk_a = C1a[rows].T                # (51, 128)
        blk_r = C1r[rows].T
        for pl in range(2):
            g1[0:51, pl, 128 * s:128 * (s + 1)] = blk_a
            g1[51:102, pl, 128 * s:128 * (s + 1)] = blk_r

    # g2: (128, 9*128) bf16 stage-B lhsT; out = lhsT.T @ t.
    # window q (a,b): lhsT[k, m<64] = W2[m, k]/deg_a;
    #                 lhsT[64+k, 64+m] = W2[m, k]/deg_b.
    # q8 (single joint 10): block at partitions 64-127 (rhs lives at hi),
    # M-cols 64-127 zero (zeroes psum hi half on start=True).
    g2 = np.zeros((128, 9 * 128), np.float32)
    for q, (a, b) in enumerate(H2WIN):
        if b is not None:
            g2[0:64, 128 * q:128 * q + 64] = W2.T / deg[a]
            g2[64:128, 128 * q + 64:128 * q + 128] = W2.T / deg[b]
        else:
            g2[64:128, 128 * q:128 * q + 64] = W2.T / deg[a]

    # g3: (128, 9*64) bf16 stage-C lhsT: lhsT[k, m] = D[m, a*64+k],
    # lhsT[64+k, m] = D[m, b*64+k]; q8 hi rows zero.
    g3 = np.zeros((128, 9 * 64), np.float32)
    for q, (a, b) in enumerate(H2WIN):
        g3[0:64, 64 * q:64 * (q + 1)] = D[:, a * H:(a + 1) * H].T
        if b is not None:
            g3[64:128, 64 * q:64 * (q + 1)] = D[:, b * H:(b + 1) * H].T

    wp2t = np.zeros((65, DOUT), np.float32)
    wp2t[0:64] = Wp2.T
    wp2t[64] = bp2

    return {
        "g1": g1.astype(NP_F8),
        "g2": g2.astype(NP_BF),
        "g3": g3.astype(NP_BF),
        "wp2t": wp2t.astype(NP_BF),
        "b1p": np.tile(b1, 2).reshape(128, 1).copy(),
        "b2p": np.tile(b2, 2).reshape(128, 1).copy(),
        "bzt": bzt.reshape(64, 1).copy(),
    }


def _pack_x(x_core):
    """x_core (n, 51) fp32 -> (102, 2, n) fp8 DR moving operand."""
    xT = np.ascontiguousarray(x_core.T)          # (51, n)
    xa = _f8(xT)
    xr = _f8(xT - xa.astype(np.float32))
    x2 = np.zeros((KA, 2, xT.shape[1]), NP_F8)
    x2[0:51, 0] = xa
    x2[51:102, 0] = xa
    x2[0:51, 1] = xr
    x2[51:102, 1] = xr
    return x2


class _Sched:
    """Greedy build-time load balancer.

    PSUM evacuations can only run on scalar (act) / vector (dve).
    SBUF-only adds can also run on gpsimd (pool).
    """
    EVAC_COST = {"act": {512: 611.0, 1024: 1038.0},
                 "dve": {512: 658.0, 1024: 1192.0}}
    AGG_COST = {"dve": 330.0, "pool": 1150.0}

    def __init__(self, nc):
        self.nc = nc
        self.load = {"act": 0.0, "dve": 0.0, "pool": 0.0}
        self.n = {"act": 0, "dve": 0, "pool": 0}

    def _pick(self, table):
        e = min(table, key=lambda k: self.load[k] + table[k])
        self.load[e] += table[e]
        self.n[e] += 1
        return e

    def evac(self, dst, src, bias, width):
        nc = self.nc
        e = self._pick({k: v[width] for k, v in self.EVAC_COST.items()})
        ALU = mybir.AluOpType
        if e == "act":
            nc.scalar.activation(dst, src, mybir.ActivationFunctionType.Relu,
                                 bias=bias)
        else:
            nc.vector.tensor_scalar(out=dst, in0=src, scalar1=bias,
                                    scalar2=0.0, op0=ALU.add, op1=ALU.max)

    def copy(self, dst, src, width):
        nc = self.nc
        e = self._pick({k: v[width] for k, v in self.EVAC_COST.items()})
        if e == "act":
            nc.scalar.copy(dst, src)
        else:
            nc.vector.tensor_copy(dst, src)

    def agg(self, out, in0, in1):
        nc = self.nc
        e = self._pick(self.AGG_COST)
        if e == "dve":
            nc.vector.tensor_tensor(out=out, in0=in0, in1=in1,
                                    op=mybir.AluOpType.add)
        else:
            nc.gpsimd.tensor_tensor(out=out, in0=in0, in1=in1,
                                    op=mybir.AluOpType.add)


def _build_program(reps=1):
    nc = bacc.Bacc(None)

    x2_d = nc.declare_dram_parameter("x2", [KA, 2, BC], F8, isOutput=False)
    g1_d = nc.declare_dram_parameter("g1", [KA, 2, 9 * 128], F8, isOutput=False)
    g2_d = nc.declare_dram_parameter("g2", [128, 9 * 128], BF16, isOutput=False)
    g3_d = nc.declare_dram_parameter("g3", [128, 9 * 64], BF16, isOutput=False)
    wp2t_d = nc.declare_dram_parameter("wp2t", [65, DOUT], BF16, isOutput=False)
    b1p_d = nc.declare_dram_parameter("b1p", [128, 1], F32, isOutput=False)
    b2p_d = nc.declare_dram_parameter("b2p", [128, 1], F32, isOutput=False)
    bzt_d = nc.declare_dram_parameter("bzt", [H, 1], F32, isOutput=False)
    out_d = nc.declare_dram_parameter("out", [BC, DOUT], BF16, isOutput=True)

    out_r = out_d.rearrange("(c p) f -> p c f", p=128)      # (128, 64, 256)

    MM = mybir.MatmulPerfMode

    with tile.TileContext(nc) as tc:
        with (
            tc.tile_pool(name="const", bufs=1) as cp,
            tc.tile_pool(name="h1", bufs=2) as h1p,
            tc.tile_pool(name="tt", bufs=2) as tp,
            tc.tile_pool(name="h7", bufs=2) as h7p,
            tc.tile_pool(name="h2", bufs=2) as h2p,
            tc.tile_pool(name="ot", bufs=3) as otp,
            tc.tile_pool(name="psm", bufs=3, space="PSUM") as psm,
            tc.tile_pool(name="psc", bufs=2, space="PSUM") as psc,
        ):
            x2_sb = cp.tile([KA, 2, BC], F8)
            g1_sb = cp.tile([KA, 2, 9 * 128], F8)
            g2_sb = cp.tile([128, 9 * 128], BF16)
            g3_sb = cp.tile([128, 9 * 64], BF16)
            wp2t_sb = cp.tile([65, DOUT], BF16)
            b1p_sb = cp.tile([128, 1], F32)
            b2p_sb = cp.tile([128, 1], F32)
            bzt_sb = cp.tile([H, 1], F32)
            z4_sb = cp.tile([65, 2 * TILE_N], BF16)   # manual double buffer

            nc.sync.dma_start(x2_sb[:, :, :], x2_d[:, :, :])
            nc.sync.dma_start(g1_sb[:, :, :], g1_d[:, :, :])
            nc.sync.dma_start(g2_sb[:], g2_d[:])
            nc.sync.dma_start(g3_sb[:], g3_d[:])
            nc.sync.dma_start(wp2t_sb[:], wp2t_d[:])
            nc.sync.dma_start(b1p_sb[:], b1p_d[:])
            nc.sync.dma_start(b2p_sb[:], b2p_d[:])
            nc.sync.dma_start(bzt_sb[:], bzt_d[:])
            nc.gpsimd.memset(z4_sb[64:65, :], 1.0)    # stage-D bias row

            sched = _Sched(nc)
            state = {}

            def a_pair(g, h1_sb, s0, nwin):
                """A matmuls for slots s0..s0+nwin-1 into one psum tile,
                one evac."""
                ps = psm.tile([128, 2 * TILE_N], F32, name="ps_m")
                for i in range(nwin):
                    s = s0 + i
                    nc.tensor.matmul(
                        ps[:, TILE_N * i:TILE_N * (i + 1)],
                        g1_sb[:, :, 128 * s:128 * (s + 1)],
                        x2_sb[:, :, TILE_N * g:TILE_N * (g + 1)],
                        start=True, stop=True,
                        perf_mode=MM.DoubleRow,
                    )
                w = nwin * TILE_N
                sched.evac(h1_sb[:, TILE_N * s0:TILE_N * s0 + w], ps[:, 0:w],
                           b1p_sb[:], w)

            def stage_a_pre(g):
                h1_sb = h1p.tile([128, 9 * TILE_N], BF16)
                h7_sb = h7p.tile([128, TILE_N], BF16)
                state[("h1", g)] = h1_sb
                state[("h7", g)] = h7_sb
                a_pair(g, h1_sb, 0, 2)        # w1, w4

            def stage_a_mid(g):
                h1_sb = state[("h1", g)]
                h7_sb = state[("h7", g)]
                a_pair(g, h1_sb, 2, 2)        # w7, w0
                # synthesize h1[7] at hi partitions via DMA (slot 2 = w7)
                nc.sync.dma_start(
                    h7_sb[64:128, :],
                    h1_sb[0:64, TILE_N * 2:TILE_N * 3],
                )

            def stage_a_post(g):
                h1_sb = state[("h1", g)]
                h7_sb = state[("h7", g)]
                a_pair(g, h1_sb, 4, 2)        # w2, w3
                a_pair(g, h1_sb, 6, 2)        # w5, w6
                a_pair(g, h1_sb, 8, 1)        # w8
                # aggregation: t windows (bf16 adds, DVE/GpSimd)
                t_sb = tp.tile([128, 6 * TILE_N], BF16)
                state[("t", g)] = t_sb

                def hw(w):
                    s = SLOT_OF[w]
                    return h1_sb[:, TILE_N * s:TILE_N * (s + 1)]

                def ts(q):
                    s = TSLOT[q]
                    return t_sb[:, TILE_N * s:TILE_N * (s + 1)]

                for q in [1, 2, 4, 5, 7]:
                    ws = TPLAN[q][1]
                    sched.agg(ts(q), hw(ws[0]), hw(ws[1]))
                # q0: 3-term chain + h7hi half (chain stays on dve)
                nc.vector.tensor_tensor(out=ts(0), in0=hw(1), in1=hw(4),
                                        op=mybir.AluOpType.add)
                nc.vector.tensor_tensor(out=ts(0), in0=ts(0), in1=hw(7),
                                        op=mybir.AluOpType.add)
                nc.vector.tensor_tensor(out=ts(0)[64:128, :],
                                        in0=ts(0)[64:128, :],
                                        in1=h7_sb[64:128, :],
                                        op=mybir.AluOpType.add)

            def b_rhs(q, h1_sb, t_sb):
                kind = TPLAN[q][0]
                if kind == "alias":
                    return h1_sb[:, TILE_N * SLOT_OF[TPLAN[q][1]]:
                                 TILE_N * (SLOT_OF[TPLAN[q][1]] + 1)]
                if kind == "alias_hi":
                    return h1_sb[64:128, TILE_N * SLOT_OF[TPLAN[q][1]]:
                                 TILE_N * (SLOT_OF[TPLAN[q][1]] + 1)]
                s = TSLOT[q]
                return t_sb[:, TILE_N * s:TILE_N * (s + 1)]

            def b_mm(q, rhs, ps, off):
                if TPLAN[q][0] == "alias_hi":
                    nc.tensor.matmul(
                        ps[:, off:off + TILE_N],
                        g2_sb[64:128, 128 * q:128 * (q + 1)],
                        rhs, start=True, stop=True,
                        tile_position=(64, 0),
                    )
                else:
                    nc.tensor.matmul(
                        ps[:, off:off + TILE_N],
                        g2_sb[:, 128 * q:128 * (q + 1)],
                        rhs, start=True, stop=True,
                    )

            def stage_b(g):
                h1_sb = state.pop(("h1", g))
                t_sb = state.pop(("t", g))
                state.pop(("h7", g))
                h2_sb = h2p.tile([128, 9 * TILE_N], BF16)
                state[("h2", g)] = h2_sb
                for q0 in (2, 4, 6, 0):   # q0's rhs (t0) is ready last
                    ps = psm.tile([128, 2 * TILE_N], F32, name="ps_m")
                    b_mm(q0, b_rhs(q0, h1_sb, t_sb), ps, 0)
                    b_mm(q0 + 1, b_rhs(q0 + 1, h1_sb, t_sb), ps, TILE_N)
                    sched.evac(h2_sb[:, TILE_N * q0:TILE_N * (q0 + 2)],
                               ps[:, :], b2p_sb[:], 2 * TILE_N)
                ps = psm.tile([128, 2 * TILE_N], F32, name="ps_m")
                b_mm(8, b_rhs(8, h1_sb, t_sb), ps, 0)
                sched.evac(h2_sb[:, TILE_N * 8:TILE_N * 9],
                           ps[:, 0:TILE_N], b2p_sb[:], TILE_N)

            def stage_c(g):
                h2_sb = state.pop(("h2", g))
                ps_c = psc.tile([128, TILE_N], F32)
                order = (2, 3, 4, 5, 6, 7, 0, 1, 8)
                for i, p in enumerate(order):
                    nc.tensor.matmul(
                        ps_c[0:64, :],
                        g3_sb[:, 64 * p:64 * (p + 1)],
                        h2_sb[:, TILE_N * p:TILE_N * (p + 1)],
                        start=(i == 0), stop=(i == len(order) - 1),
                    )
                zoff = TILE_N * (g % 2)
                sched.evac(z4_sb[0:64, zoff:zoff + TILE_N], ps_c[0:64, :],
                           bzt_sb[:], TILE_N)

            def stage_d(g):
                zoff = TILE_N * (g % 2)
                ot_sb = otp.tile([128, 4 * DOUT], BF16)
                ps_d = psm.tile([128, 2 * TILE_N], F32, name="ps_m")
                for kk in range(4):
                    nc.tensor.matmul(
                        ps_d[:, DOUT * kk:DOUT * (kk + 1)],
                        z4_sb[0:65, zoff + 128 * kk:zoff + 128 * (kk + 1)],
                        wp2t_sb[:],
                        start=True, stop=True,
                    )
                sched.copy(ot_sb[:], ps_d[:], 2 * TILE_N)
                nc.sync.dma_start(out_r[:, 4 * g:4 * (g + 1), :], ot_sb[:])

            ntiles = int(os.environ.get("KERNEL_NTILES", str(NTILES)))

            def body():
                # deep pipeline: every stage consumes inputs >= 1 iteration
                # old, so DMA/agg chains never sit on the critical path.
                for g in range(ntiles + 3):
                    if g < ntiles:
                        stage_a_pre(g)
                        stage_a_mid(g)
                        stage_a_post(g)
                    if 1 <= g <= ntiles:
                        stage_b(g - 1)
                    if 2 <= g <= ntiles + 1:
                        stage_c(g - 2)
                    if 3 <= g:
                        stage_d(g - 3)

            if reps == 1:
                body()
            else:
                with tc.For_i(0, reps, 1):
                    body()

    nc.compile()
    return nc


_CACHE = {}


def kernel(**inputs):
    global LAST_RESULTS
    from concourse.bass_utils import run_bass_kernel_spmd

    x = np.ascontiguousarray(np.asarray(inputs["x"], np.float32))
    consts = _build_constants(
        inputs["A"], inputs["W1"], inputs["b1"], inputs["W2"], inputs["b2"],
        inputs["W3"], inputs["b3"], inputs["Wp1"], inputs["bp1"],
        inputs["Wp2"], inputs["bp2"],
    )

    reps = int(os.environ.get("BENCH_REPS", "1"))
    key = (reps,)
    if key not in _CACHE:
        _CACHE[key] = _build_program(reps=reps)
    nc = _CACHE[key]

    xf = x.reshape(B_TOTAL, NJ * DIN)
    in_maps = []
    for c in range(NCORES):
        m = dict(consts)
        m["x2"] = _pack_x(xf[c * BC:(c + 1) * BC])
        in_maps.append(m)

    res = run_bass_kernel_spmd(nc, in_maps, list(range(NCORES)))
    LAST_RESULTS = res
    out = np.concatenate([np.asarray(res.results[c]["out"])
                          for c in range(NCORES)], axis=0)
    return out.astype(np.float32)


if __name__ == "__main__":
    import reference
    ins = {k: np.asarray(v) for k, v in reference.setup_inputs().items()}
    exp = np.asarray(reference.reference(**reference.setup_inputs()))
    os.environ.setdefault("BENCH_REPS", "1")
    act = kernel(**ins)
    scale = float(np.abs(exp).max())
    err = float(np.abs(act - exp).max())
    print(f"scale={scale:.4f} abserr={err:.3e} relerr={err / scale:.3e}")


# revision 18
# speedup vs baseline: 1.5268x; 1.1947x over previous
"""Trainium2 Bass kernel for the GCNEncoder problem (v3.2).

Strategy (window dataflow, minimal PE + ALU work):
  - Pure data parallelism: batch 65536 = 8192 per core x 8 cores.
  - Stage A (h1): 9 windows, fp8e4 DoubleRow matmuls with a 2-term (value +
    residual) fp8 expansion of both x and C1 = kron(A,W1): full 4-term
    product in one DR matmul per window (256 PE cycles instead of 512).
  - Aggregation t = (unnormalized) neighbor-sum of h1 done with 8 bf16
    tensor_tensor adds on DVE/GpSimd; window pairings chosen (exact search)
    so each t-window is a sum of full 128-partition h1 windows; joint 7's
    hi-copy is synthesized by an idle SBUF->SBUF DMA.
  - Stage B (h2): one bf16 matmul per window, lhsT = blockdiag(W2/deg_a,
    W2/deg_b) (deg-normalization folded into the weights).
  - Stage C (z): 9 accumulating bf16 matmuls vs D = Wp1 @ kron(A,W3).
  - Stage D: K=65 (bias row), 4 matmuls N=256 into one (128,1024) psum.
  - GPSIMD cannot touch PSUM, so all PSUM evacuations run on ScalarE/
    VectorE as (128,1024) double-window ops where possible (shared 2-bank
    psum tiles), balanced by a greedy build-time scheduler.
"""

import os
import sys

for _p in ("/opt/trn_rl_repo", "/root/.axon_site/_ro/trn_rl_repo"):
    if os.path.isdir(_p) and _p not in sys.path:
        sys.path.insert(0, _p)

import numpy as np
import ml_dtypes

from concourse import bacc, mybir, tile

NJ = 17
DIN = 3
H = 64
DOUT = 256
NCORES = 8
B_TOTAL = 65536
BC = B_TOTAL // NCORES          # 8192
TILE_N = 512
NTILES = BC // TILE_N           # 16
KA = 102                        # stage-A DR partition count (2*51)

F32 = mybir.dt.float32
BF16 = mybir.dt.bfloat16
F8 = mybir.dt.float8e4
NP_F8 = ml_dtypes.float8_e4m3
NP_BF = ml_dtypes.bfloat16

CONNS = [(0, 7), (7, 8), (8, 9), (9, 10), (0, 1), (1, 2), (2, 3), (0, 4),
         (4, 5), (5, 6), (8, 11), (11, 12), (12, 13), (8, 14), (14, 15), (15, 16)]

# h1 window w = (lo joint, hi joint); from exact pairing search.
H1WIN = [(0, 8), (1, 11), (2, 12), (3, 13), (4, 14), (5, 15), (6, 16), (7, 9),
         (8, 10)]
# h1 slot order in h1_sb (so evac pairs are column-adjacent); t0 chain's
# inputs (w1, w4, w7) come first.
A_ORDER = [1, 4, 7, 0, 2, 3, 5, 6, 8]
SLOT_OF = {w: s for s, w in enumerate(A_ORDER)}

# t/h2 window q = (lo joint, hi joint); q8 holds joint 10 at lo only.
H2WIN = [(0, 8), (1, 11), (2, 12), (3, 13), (4, 14), (5, 15), (6, 16), (7, 9),
         (10, None)]
# t-window construction: q -> ("alias", h1win) / ("alias_hi", h1win) /
# ("sum", [h1 windows]) / ("edge", [h1 windows]).
# "sum": DVE adds of full h1 windows, then one blockdiag matmul.
# "edge": PE accumulation - one matmul per h1 window with the SAME blockdiag
#   lhsT (valid because lo->lo / hi->hi alignment); q0's w7 operand needs a
#   special cross block (joint 7 feeds both outputs) stored in g2e.
TPLAN = {
    0: ("edge", [1, 4, 7]),      # w7 via g2e cross block
    1: ("sum", [0, 2]),
    2: ("sum", [1, 3]),
    3: ("alias", 2),
    4: ("sum", [0, 5]),
    5: ("sum", [4, 6]),
    6: ("alias", 5),
    7: ("sum", [0, 8]),
    8: ("alias_hi", 7),          # rhs = h1 win 7 hi half = h1[9]; N(10)={9}
}
# t slots in t_sb for the "sum" windows
TSLOT = {1: 0, 2: 1, 4: 2, 5: 3, 7: 4}
N_TSLOT = 5

LAST_RESULTS = None


def _neighbors():
    nbr = [set() for _ in range(NJ)]
    for a, b in CONNS:
        nbr[a].add(b)
        nbr[b].add(a)
    return nbr


def _f8(a):
    return np.asarray(a, np.float32).astype(NP_F8)


def _build_constants(A, W1, b1, W2, b2, W3, b3, Wp1, bp1, Wp2, bp2):
    A = np.asarray(A, np.float32)
    W1 = np.asarray(W1, np.float32)
    W2 = np.asarray(W2, np.float32)
    W3 = np.asarray(W3, np.float32)
    Wp1 = np.asarray(Wp1, np.float32)
    Wp2 = np.asarray(Wp2, np.float32)
    b1 = np.asarray(b1, np.float32)
    b2 = np.asarray(b2, np.float32)
    b3 = np.asarray(b3, np.float32)
    bp1 = np.asarray(bp1, np.float32)
    bp2 = np.asarray(bp2, np.float32)
    nbr = _neighbors()
    deg = np.array([len(nbr[j]) for j in range(NJ)], np.float32)

    C1 = np.kron(A, W1)                    # (1088, 51)
    C3 = np.kron(A, W3)
    D = Wp1 @ C3                           # (64, 1088)
    bzt = Wp1 @ np.tile(b3, NJ) + bp1      # (64,)

    C1a = _f8(C1).astype(np.float32)
    C1r = _f8(C1 - C1a).astype(np.float32)

    # g1: (102, 2, 9*128) fp8 DR lhsT (indexed by h1 SLOT, not window id).
    #   both planes rows 0-50 = C1a[rows_w].T, rows 51-101 = C1r[rows_w].T;
    #   paired with x planes (xa; xa) and (xr; xr): full 4-term product.
    g1 = np.zeros((KA, 2, 9 * 128), np.float32)
    for w, (u, v) in enumerate(H1WIN):
        s = SLOT_OF[w]
        rows = np.concatenate([np.arange(u * H, (u + 1) * H),
                               np.arange(v * H, (v + 1) * H)])
        blk_a = C1a[rows].T                # (51, 128)
        blk_r = C1r[rows].T
        for pl in range(2):
            g1[0:51, pl, 128 * s:128 * (s + 1)] = blk_a
            g1[51:102, pl, 128 * s:128 * (s + 1)] = blk_r

    # g2: (128, 9*128) bf16 stage-B lhsT; out = lhsT.T @ t.
    # window q (a,b): lhsT[k, m<64] = W2[m, k]/deg_a;
    #                 lhsT[64+k, 64+m] = W2[m, k]/deg_b.
    # q8 (single joint 10): block at partitions 64-127 (rhs lives at hi),
    # M-cols 64-127 zero (zeroes psum hi half on start=True).
    g2 = np.zeros((128, 9 * 128), np.float32)
    for q, (a, b) in enumerate(H2WIN):
        if b is not None:
            g2[0:64, 128 * q:128 * q + 64] = W2.T / deg[a]
            g2[64:128, 128 * q + 64:128 * q + 128] = W2.T / deg[b]
        else:
            g2[64:128, 128 * q:128 * q + 64] = W2.T / deg[a]

    # g3: (128, 9*64) bf16 stage-C lhsT: lhsT[k, m] = D[m, a*64+k],
    # lhsT[64+k, m] = D[m, b*64+k]; q8 hi rows zero.
    g3 = np.zeros((128, 9 * 64), np.float32)
    for q, (a, b) in enumerate(H2WIN):
        g3[0:64, 64 * q:64 * (q + 1)] = D[:, a * H:(a + 1) * H].T
        if b is not None:
            g3[64:128, 64 * q:64 * (q + 1)] = D[:, b * H:(b + 1) * H].T

    # g2e: (128, 128) bf16, q0's lhsT for operand window w7=(7,9):
    # joint 7 (lo rows) feeds joint 0 (lo out, /deg0) AND joint 8 (hi out,
    # /deg8); joint 9 (hi rows) feeds joint 8 (hi out, /deg8).
    g2e = np.zeros((128, 128), np.float32)
    g2e[0:64, 0:64] = W2.T / deg[0]
    g2e[0:64, 64:128] = W2.T / deg[8]
    g2e[64:128, 64:128] = W2.T / deg[8]

    wp2t = np.zeros((65, DOUT), np.float32)
    wp2t[0:64] = Wp2.T
    wp2t[64] = bp2

    return {
        "g1": g1.astype(NP_F8),
        "g2": g2.astype(NP_BF),
        "g2e": g2e.astype(NP_BF),
        "g3": g3.astype(NP_BF),
        "wp2t": wp2t.astype(NP_BF),
        "b1p": np.tile(b1, 2).reshape(128, 1).copy(),
        "b2p": np.tile(b2, 2).reshape(128, 1).copy(),
        "bzt": bzt.reshape(64, 1).copy(),
    }


def _pack_x(x_core):
    """x_core (n, 51) fp32 -> (102, 2, n) fp8 DR moving operand."""
    xT = np.ascontiguousarray(x_core.T)          # (51, n)
    xa = _f8(xT)
    xr = _f8(xT - xa.astype(np.float32))
    x2 = np.zeros((KA, 2, xT.shape[1]), NP_F8)
    x2[0:51, 0] = xa
    x2[51:102, 0] = xa
    x2[0:51, 1] = xr
    x2[51:102, 1] = xr
    return x2


class _Sched:
    """Greedy build-time load balancer.

    PSUM evacuations can only run on scalar (act) / vector (dve).
    SBUF-only adds can also run on gpsimd (pool).
    """
    EVAC_COST = {"act": {512: 611.0, 1024: 1038.0},
                 "dve": {512: 658.0, 1024: 1192.0}}
    AGG_COST = {"dve": 330.0, "pool": 1150.0}

    def __init__(self, nc):
        self.nc = nc
        self.load = {"act": 0.0, "dve": 0.0, "pool": 0.0}
        self.n = {"act": 0, "dve": 0, "pool": 0}

    def _pick(self, table):
        e = min(table, key=lambda k: self.load[k] + table[k])
        self.load[e] += table[e]
        self.n[e] += 1
        return e

    def evac(self, dst, src, bias, width):
        nc = self.nc
        e = self._pick({k: v[width] for k, v in self.EVAC_COST.items()})
        ALU = mybir.AluOpType
        if e == "act":
            nc.scalar.activation(dst, src, mybir.ActivationFunctionType.Relu,
                                 bias=bias)
        else:
            nc.vector.tensor_scalar(out=dst, in0=src, scalar1=bias,
                                    scalar2=0.0, op0=ALU.add, op1=ALU.max)

    def copy(self, dst, src, width):
        nc = self.nc
        e = self._pick({k: v[width] for k, v in self.EVAC_COST.items()})
        if e == "act":
            nc.scalar.copy(dst, src)
        else:
            nc.vector.tensor_copy(dst, src)

    def agg(self, out, in0, in1):
        nc = self.nc
        e = self._pick(self.AGG_COST)
        if e == "dve":
            nc.vector.tensor_tensor(out=out, in0=in0, in1=in1,
                                    op=mybir.AluOpType.add)
        else:
            nc.gpsimd.tensor_tensor(out=out, in0=in0, in1=in1,
                                    op=mybir.AluOpType.add)


def _build_program(reps=1):
    pe_only = bool(int(os.environ.get("KERNEL_PE_ONLY", "0")))
    alu_only = bool(int(os.environ.get("KERNEL_ALU_ONLY", "0")))
    edge_full = os.environ.get("KERNEL_EDGE", "full") == "full"
    tplan = dict(TPLAN)
    if edge_full:
        for q in [1, 2, 4, 5, 7]:
            tplan[q] = ("edge", TPLAN[q][1])
    nc = bacc.Bacc(None)

    x2_d = nc.declare_dram_parameter("x2", [KA, 2, BC], F8, isOutput=False)
    g1_d = nc.declare_dram_parameter("g1", [KA, 2, 9 * 128], F8, isOutput=False)
    g2_d = nc.declare_dram_parameter("g2", [128, 9 * 128], BF16, isOutput=False)
    g2e_d = nc.declare_dram_parameter("g2e", [128, 128], BF16, isOutput=False)
    g3_d = nc.declare_dram_parameter("g3", [128, 9 * 64], BF16, isOutput=False)
    wp2t_d = nc.declare_dram_parameter("wp2t", [65, DOUT], BF16, isOutput=False)
    b1p_d = nc.declare_dram_parameter("b1p", [128, 1], F32, isOutput=False)
    b2p_d = nc.declare_dram_parameter("b2p", [128, 1], F32, isOutput=False)
    bzt_d = nc.declare_dram_parameter("bzt", [H, 1], F32, isOutput=False)
    # partition-major output: each partition writes contiguous 2KB per tile;
    # host permutes back to (BC, DOUT).
    out_d = nc.declare_dram_parameter("out", [128, BC // 128, DOUT], BF16,
                                      isOutput=True)
    out_r = out_d

    MM = mybir.MatmulPerfMode

    with tile.TileContext(nc) as tc:
        with (
            tc.tile_pool(name="const", bufs=1) as cp,
            tc.tile_pool(name="h1", bufs=2) as h1p,
            tc.tile_pool(name="tt", bufs=2) as tp,
            tc.tile_pool(name="h2", bufs=2) as h2p,
            tc.tile_pool(name="ot", bufs=3) as otp,
            tc.tile_pool(name="psm", bufs=3, space="PSUM") as psm,
            tc.tile_pool(name="psc", bufs=2, space="PSUM") as psc,
        ):
            x2_sb = cp.tile([KA, 2, BC], F8)
            g1_sb = cp.tile([KA, 2, 9 * 128], F8)
            g2_sb = cp.tile([128, 9 * 128], BF16)
            g2e_sb = cp.tile([128, 128], BF16)
            g3_sb = cp.tile([128, 9 * 64], BF16)
            wp2t_sb = cp.tile([65, DOUT], BF16)
            b1p_sb = cp.tile([128, 1], F32)
            b2p_sb = cp.tile([128, 1], F32)
            bzt_sb = cp.tile([H, 1], F32)
            z4_sb = cp.tile([65, 2 * TILE_N], BF16)   # manual double buffer

            nc.sync.dma_start(x2_sb[:, :, :], x2_d[:, :, :])
            nc.sync.dma_start(g1_sb[:, :, :], g1_d[:, :, :])
            nc.sync.dma_start(g2_sb[:], g2_d[:])
            nc.sync.dma_start(g2e_sb[:], g2e_d[:])
            nc.sync.dma_start(g3_sb[:], g3_d[:])
            nc.sync.dma_start(wp2t_sb[:], wp2t_d[:])
            nc.sync.dma_start(b1p_sb[:], b1p_d[:])
            nc.sync.dma_start(b2p_sb[:], b2p_d[:])
            nc.sync.dma_start(bzt_sb[:], bzt_d[:])
            nc.gpsimd.memset(z4_sb[64:65, :], 1.0)    # stage-D bias row

            sched = _Sched(nc)
            state = {}

            def a_pair(g, h1_sb, s0, nwin):
                """A matmuls for slots s0..s0+nwin-1 into one psum tile,
                one evac."""
                ps = psm.tile([128, 2 * TILE_N], F32, name="ps_m")
                if not alu_only:
                    for i in range(nwin):
                        s = s0 + i
                        nc.tensor.matmul(
                            ps[:, TILE_N * i:TILE_N * (i + 1)],
                            g1_sb[:, :, 128 * s:128 * (s + 1)],
                            x2_sb[:, :, TILE_N * g:TILE_N * (g + 1)],
                            start=True, stop=True,
                            perf_mode=MM.DoubleRow,
                        )
                w = nwin * TILE_N
                if not pe_only:
                    sched.evac(h1_sb[:, TILE_N * s0:TILE_N * s0 + w],
                               ps[:, 0:w], b1p_sb[:], w)

            def stage_a_pre(g):
                h1_sb = h1p.tile([128, 9 * TILE_N], BF16)
                state[("h1", g)] = h1_sb
                a_pair(g, h1_sb, 0, 2)        # w1, w4

            def stage_a_mid(g):
                h1_sb = state[("h1", g)]
                a_pair(g, h1_sb, 2, 2)        # w7, w0

            def stage_a_post(g):
                h1_sb = state[("h1", g)]
                a_pair(g, h1_sb, 4, 2)        # w2, w3
                a_pair(g, h1_sb, 6, 2)        # w5, w6
                a_pair(g, h1_sb, 8, 1)        # w8
                # aggregation: t windows (bf16 adds, DVE/GpSimd)
                t_sb = tp.tile([128, N_TSLOT * TILE_N], BF16)
                state[("t", g)] = t_sb

                def hw(w):
                    s = SLOT_OF[w]
                    return h1_sb[:, TILE_N * s:TILE_N * (s + 1)]

                def ts(q):
                    s = TSLOT[q]
                    return t_sb[:, TILE_N * s:TILE_N * (s + 1)]

                if not pe_only:
                    for q in [1, 2, 4, 5, 7]:
                        if tplan[q][0] != "sum":
                            continue
                        ws = tplan[q][1]
                        sched.agg(ts(q), hw(ws[0]), hw(ws[1]))

            def hw_of(h1_sb, w):
                sl = SLOT_OF[w]
                return h1_sb[:, TILE_N * sl:TILE_N * (sl + 1)]

            def b_rhs(q, h1_sb, t_sb):
                kind = tplan[q][0]
                if kind == "alias":
                    return hw_of(h1_sb, tplan[q][1])
                if kind == "alias_hi":
                    return hw_of(h1_sb, tplan[q][1])[64:128, :]
                sl = TSLOT[q]
                return t_sb[:, TILE_N * sl:TILE_N * (sl + 1)]

            def b_mm(q, h1_sb, t_sb, ps, off):
                kind = tplan[q][0]
                if kind == "edge":
                    ws = tplan[q][1]
                    for i, w in enumerate(ws):
                        lhsT = (g2e_sb[:, :] if w == 7
                                else g2_sb[:, 128 * q:128 * (q + 1)])
                        nc.tensor.matmul(
                            ps[:, off:off + TILE_N], lhsT,
                            hw_of(h1_sb, w),
                            start=(i == 0), stop=(i == len(ws) - 1),
                        )
                elif kind == "alias_hi":
                    nc.tensor.matmul(
                        ps[:, off:off + TILE_N],
                        g2_sb[64:128, 128 * q:128 * (q + 1)],
                        b_rhs(q, h1_sb, t_sb), start=True, stop=True,
                        tile_position=(64, 0),
                    )
                else:
                    nc.tensor.matmul(
                        ps[:, off:off + TILE_N],
                        g2_sb[:, 128 * q:128 * (q + 1)],
                        b_rhs(q, h1_sb, t_sb), start=True, stop=True,
                    )

            def stage_b(g):
                h1_sb = state.pop(("h1", g))
                t_sb = state.pop(("t", g))
                h2_sb = h2p.tile([128, 9 * TILE_N], BF16)
                state[("h2", g)] = h2_sb
                for q0 in (2, 4, 6, 0):
                    ps = psm.tile([128, 2 * TILE_N], F32, name="ps_m")
                    if not alu_only:
                        b_mm(q0, h1_sb, t_sb, ps, 0)
                        b_mm(q0 + 1, h1_sb, t_sb, ps, TILE_N)
                    if not pe_only:
                        sched.evac(h2_sb[:, TILE_N * q0:TILE_N * (q0 + 2)],
                                   ps[:, :], b2p_sb[:], 2 * TILE_N)
                ps = psm.tile([128, 2 * TILE_N], F32, name="ps_m")
                if not alu_only:
                    b_mm(8, h1_sb, t_sb, ps, 0)
                if not pe_only:
                    sched.evac(h2_sb[:, TILE_N * 8:TILE_N * 9],
                               ps[:, 0:TILE_N], b2p_sb[:], TILE_N)

            def stage_c(g):
                h2_sb = state.pop(("h2", g))
                ps_c = psc.tile([128, TILE_N], F32)
                order = (2, 3, 4, 5, 6, 7, 0, 1, 8)
                if not alu_only:
                    for i, p in enumerate(order):
                        nc.tensor.matmul(
                            ps_c[0:64, :],
                            g3_sb[:, 64 * p:64 * (p + 1)],
                            h2_sb[:, TILE_N * p:TILE_N * (p + 1)],
                            start=(i == 0), stop=(i == len(order) - 1),
                        )
                zoff = TILE_N * (g % 2)
                if not pe_only:
                    sched.evac(z4_sb[0:64, zoff:zoff + TILE_N], ps_c[0:64, :],
                               bzt_sb[:], TILE_N)

            def stage_d(g):
                zoff = TILE_N * (g % 2)
                ot_sb = otp.tile([128, 4 * DOUT], BF16)
                ps_d = psm.tile([128, 2 * TILE_N], F32, name="ps_m")
                if not alu_only:
                    for kk in range(4):
                        nc.tensor.matmul(
                            ps_d[:, DOUT * kk:DOUT * (kk + 1)],
                            z4_sb[0:65, zoff + 128 * kk:zoff + 128 * (kk + 1)],
                            wp2t_sb[:],
                            start=True, stop=True,
                        )
                if not pe_only:
                    sched.copy(ot_sb[:], ps_d[:], 2 * TILE_N)
                if not (pe_only or alu_only):
                    nc.sync.dma_start(out_r[:, 4 * g:4 * (g + 1), :], ot_sb[:])

            ntiles = int(os.environ.get("KERNEL_NTILES", str(NTILES)))

            def body():
                # deep pipeline: every stage consumes inputs >= 1 iteration
                # old, so DMA/agg chains never sit on the critical path.
                for g in range(ntiles + 3):
                    if g < ntiles:
                        stage_a_pre(g)
                        stage_a_mid(g)
                        stage_a_post(g)
                    if 1 <= g <= ntiles:
                        stage_b(g - 1)
                    if 2 <= g <= ntiles + 1:
                        stage_c(g - 2)
                    if 3 <= g:
                        stage_d(g - 3)

            if reps == 1:
                body()
            else:
                with tc.For_i(0, reps, 1):
                    body()

    nc.compile()
    return nc


_CACHE = {}


def kernel(**inputs):
    global LAST_RESULTS
    from concourse.bass_utils import run_bass_kernel_spmd

    x = np.ascontiguousarray(np.asarray(inputs["x"], np.float32))
    consts = _build_constants(
        inputs["A"], inputs["W1"], inputs["b1"], inputs["W2"], inputs["b2"],
        inputs["W3"], inputs["b3"], inputs["Wp1"], inputs["bp1"],
        inputs["Wp2"], inputs["bp2"],
    )

    reps = int(os.environ.get("BENCH_REPS", "1"))
    key = (reps,)
    if key not in _CACHE:
        _CACHE[key] = _build_program(reps=reps)
    nc = _CACHE[key]

    xf = x.reshape(B_TOTAL, NJ * DIN)
    in_maps = []
    for c in range(NCORES):
        m = dict(consts)
        m["x2"] = _pack_x(xf[c * BC:(c + 1) * BC])
        in_maps.append(m)

    res = run_bass_kernel_spmd(nc, in_maps, list(range(NCORES)))
    LAST_RESULTS = res
    outs = []
    for c in range(NCORES):
        o = np.asarray(res.results[c]["out"])        # (128, 64, 256)
        outs.append(np.ascontiguousarray(o.transpose(1, 0, 2))
                    .reshape(BC, DOUT))              # row c*128+p = o[p, c]
    return np.concatenate(outs, axis=0).astype(np.float32)


if __name__ == "__main__":
    import reference
    ins = {k: np.asarray(v) for k, v in reference.setup_inputs().items()}
    exp = np.asarray(reference.reference(**reference.setup_inputs()))
    os.environ.setdefault("BENCH_REPS", "1")
    act = kernel(**ins)
    scale = float(np.abs(exp).max())
    err = float(np.abs(act - exp).max())
    print(f"scale={scale:.4f} abserr={err:.3e} relerr={err / scale:.3e}")


# revision 19
# speedup vs baseline: 1.5426x; 1.0104x over previous
"""Trainium2 Bass kernel for the GCNEncoder problem (v3.2).

Strategy (window dataflow, minimal PE + ALU work):
  - Pure data parallelism: batch 65536 = 8192 per core x 8 cores.
  - Stage A (h1): 9 windows, fp8e4 DoubleRow matmuls with a 2-term (value +
    residual) fp8 expansion of both x and C1 = kron(A,W1): full 4-term
    product in one DR matmul per window (256 PE cycles instead of 512).
  - Aggregation t = (unnormalized) neighbor-sum of h1 done with 8 bf16
    tensor_tensor adds on DVE/GpSimd; window pairings chosen (exact search)
    so each t-window is a sum of full 128-partition h1 windows; joint 7's
    hi-copy is synthesized by an idle SBUF->SBUF DMA.
  - Stage B (h2): one bf16 matmul per window, lhsT = blockdiag(W2/deg_a,
    W2/deg_b) (deg-normalization folded into the weights).
  - Stage C (z): 9 accumulating bf16 matmuls vs D = Wp1 @ kron(A,W3).
  - Stage D: K=65 (bias row), 4 matmuls N=256 into one (128,1024) psum.
  - GPSIMD cannot touch PSUM, so all PSUM evacuations run on ScalarE/
    VectorE as (128,1024) double-window ops where possible (shared 2-bank
    psum tiles), balanced by a greedy build-time scheduler.
"""

import os
import sys

for _p in ("/opt/trn_rl_repo", "/root/.axon_site/_ro/trn_rl_repo"):
    if os.path.isdir(_p) and _p not in sys.path:
        sys.path.insert(0, _p)

import numpy as np
import ml_dtypes

from concourse import bacc, mybir, tile

NJ = 17
DIN = 3
H = 64
DOUT = 256
NCORES = 8
B_TOTAL = 65536
BC = B_TOTAL // NCORES          # 8192
TILE_N = 512
NTILES = BC // TILE_N           # 16
KA = 102                        # stage-A DR partition count (2*51)

F32 = mybir.dt.float32
BF16 = mybir.dt.bfloat16
F8 = mybir.dt.float8e4
NP_F8 = ml_dtypes.float8_e4m3
NP_BF = ml_dtypes.bfloat16

CONNS = [(0, 7), (7, 8), (8, 9), (9, 10), (0, 1), (1, 2), (2, 3), (0, 4),
         (4, 5), (5, 6), (8, 11), (11, 12), (12, 13), (8, 14), (14, 15), (15, 16)]

# h1 window w = (lo joint, hi joint); from exact pairing search.
H1WIN = [(0, 8), (1, 11), (2, 12), (3, 13), (4, 14), (5, 15), (6, 16), (7, 9),
         (8, 10)]
# h1 slot order in h1_sb (so evac pairs are column-adjacent); t0 chain's
# inputs (w1, w4, w7) come first.
A_ORDER = [1, 4, 7, 0, 2, 3, 5, 6, 8]
SLOT_OF = {w: s for s, w in enumerate(A_ORDER)}

# t/h2 window q = (lo joint, hi joint); q8 holds joint 10 at lo only.
H2WIN = [(0, 8), (1, 11), (2, 12), (3, 13), (4, 14), (5, 15), (6, 16), (7, 9),
         (10, None)]
# t-window construction: q -> ("alias", h1win) / ("alias_hi", h1win) /
# ("sum", [h1 windows]) / ("edge", [h1 windows]).
# "sum": DVE adds of full h1 windows, then one blockdiag matmul.
# "edge": PE accumulation - one matmul per h1 window with the SAME blockdiag
#   lhsT (valid because lo->lo / hi->hi alignment); q0's w7 operand needs a
#   special cross block (joint 7 feeds both outputs) stored in g2e.
TPLAN = {
    0: ("edge", [1, 4, 7]),      # w7 via g2e cross block
    1: ("sum", [0, 2]),
    2: ("sum", [1, 3]),
    3: ("alias", 2),
    4: ("sum", [0, 5]),
    5: ("sum", [4, 6]),
    6: ("alias", 5),
    7: ("sum", [0, 8]),
    8: ("alias_hi", 7),          # rhs = h1 win 7 hi half = h1[9]; N(10)={9}
}
# t slots in t_sb for the "sum" windows
TSLOT = {1: 0, 2: 1, 4: 2, 5: 3, 7: 4}
N_TSLOT = 5

LAST_RESULTS = None


def _neighbors():
    nbr = [set() for _ in range(NJ)]
    for a, b in CONNS:
        nbr[a].add(b)
        nbr[b].add(a)
    return nbr


def _f8(a):
    return np.asarray(a, np.float32).astype(NP_F8)


def _build_constants(A, W1, b1, W2, b2, W3, b3, Wp1, bp1, Wp2, bp2):
    A = np.asarray(A, np.float32)
    W1 = np.asarray(W1, np.float32)
    W2 = np.asarray(W2, np.float32)
    W3 = np.asarray(W3, np.float32)
    Wp1 = np.asarray(Wp1, np.float32)
    Wp2 = np.asarray(Wp2, np.float32)
    b1 = np.asarray(b1, np.float32)
    b2 = np.asarray(b2, np.float32)
    b3 = np.asarray(b3, np.float32)
    bp1 = np.asarray(bp1, np.float32)
    bp2 = np.asarray(bp2, np.float32)
    nbr = _neighbors()
    deg = np.array([len(nbr[j]) for j in range(NJ)], np.float32)

    C1 = np.kron(A, W1)                    # (1088, 51)
    C3 = np.kron(A, W3)
    D = Wp1 @ C3                           # (64, 1088)
    bzt = Wp1 @ np.tile(b3, NJ) + bp1      # (64,)

    C1a = _f8(C1).astype(np.float32)
    C1r = _f8(C1 - C1a).astype(np.float32)

    # g1: (102, 2, 9*128) fp8 DR lhsT (indexed by h1 SLOT, not window id).
    #   both planes rows 0-50 = C1a[rows_w].T, rows 51-101 = C1r[rows_w].T;
    #   paired with x planes (xa; xa) and (xr; xr): full 4-term product.
    g1 = np.zeros((KA, 2, 9 * 128), np.float32)
    for w, (u, v) in enumerate(H1WIN):
        s = SLOT_OF[w]
        rows = np.concatenate([np.arange(u * H, (u + 1) * H),
                               np.arange(v * H, (v + 1) * H)])
        blk_a = C1a[rows].T                # (51, 128)
        blk_r = C1r[rows].T
        for pl in range(2):
            g1[0:51, pl, 128 * s:128 * (s + 1)] = blk_a
            g1[51:102, pl, 128 * s:128 * (s + 1)] = blk_r

    # g2: (128, 9*128) bf16 stage-B lhsT; out = lhsT.T @ t.
    # window q (a,b): lhsT[k, m<64] = W2[m, k]/deg_a;
    #                 lhsT[64+k, 64+m] = W2[m, k]/deg_b.
    # q8 (single joint 10): block at partitions 64-127 (rhs lives at hi),
    # M-cols 64-127 zero (zeroes psum hi half on start=True).
    g2 = np.zeros((128, 9 * 128), np.float32)
    for q, (a, b) in enumerate(H2WIN):
        if b is not None:
            g2[0:64, 128 * q:128 * q + 64] = W2.T / deg[a]
            g2[64:128, 128 * q + 64:128 * q + 128] = W2.T / deg[b]
        else:
            g2[64:128, 128 * q:128 * q + 64] = W2.T / deg[a]

    # g3: (128, 9*64) bf16 stage-C lhsT: lhsT[k, m] = D[m, a*64+k],
    # lhsT[64+k, m] = D[m, b*64+k]; q8 hi rows zero.
    g3 = np.zeros((128, 9 * 64), np.float32)
    for q, (a, b) in enumerate(H2WIN):
        g3[0:64, 64 * q:64 * (q + 1)] = D[:, a * H:(a + 1) * H].T
        if b is not None:
            g3[64:128, 64 * q:64 * (q + 1)] = D[:, b * H:(b + 1) * H].T

    # g2e: (128, 128) bf16, q0's lhsT for operand window w7=(7,9):
    # joint 7 (lo rows) feeds joint 0 (lo out, /deg0) AND joint 8 (hi out,
    # /deg8); joint 9 (hi rows) feeds joint 8 (hi out, /deg8).
    g2e = np.zeros((128, 128), np.float32)
    g2e[0:64, 0:64] = W2.T / deg[0]
    g2e[0:64, 64:128] = W2.T / deg[8]
    g2e[64:128, 64:128] = W2.T / deg[8]

    wp2t = np.zeros((65, DOUT), np.float32)
    wp2t[0:64] = Wp2.T
    wp2t[64] = bp2

    return {
        "g1": g1.astype(NP_F8),
        "g2": g2.astype(NP_BF),
        "g2e": g2e.astype(NP_BF),
        "g3": g3.astype(NP_BF),
        "wp2t": wp2t.astype(NP_BF),
        "b1p": np.tile(b1, 2).reshape(128, 1).copy(),
        "b2p": np.tile(b2, 2).reshape(128, 1).copy(),
        "bzt": bzt.reshape(64, 1).copy(),
    }


def _pack_x(x_core):
    """x_core (n, 51) fp32 -> (102, 2, n) fp8 DR moving operand."""
    xT = np.ascontiguousarray(x_core.T)          # (51, n)
    xa = _f8(xT)
    xr = _f8(xT - xa.astype(np.float32))
    x2 = np.zeros((KA, 2, xT.shape[1]), NP_F8)
    x2[0:51, 0] = xa
    x2[51:102, 0] = xa
    x2[0:51, 1] = xr
    x2[51:102, 1] = xr
    return x2


class _Sched:
    """Greedy build-time load balancer.

    PSUM evacuations can only run on scalar (act) / vector (dve).
    SBUF-only adds can also run on gpsimd (pool).
    """
    EVAC_COST = {"act": {512: 611.0, 1024: 1038.0},
                 "dve": {512: 658.0, 1024: 1192.0}}
    AGG_COST = {"dve": 330.0, "pool": 1150.0}

    def __init__(self, nc):
        self.nc = nc
        self.load = {"act": 0.0, "dve": 0.0, "pool": 0.0}
        self.n = {"act": 0, "dve": 0, "pool": 0}

    def _pick(self, table):
        e = min(table, key=lambda k: self.load[k] + table[k])
        self.load[e] += table[e]
        self.n[e] += 1
        return e

    def evac(self, dst, src, bias, width):
        nc = self.nc
        e = self._pick({k: v[width] for k, v in self.EVAC_COST.items()})
        ALU = mybir.AluOpType
        if e == "act":
            nc.scalar.activation(dst, src, mybir.ActivationFunctionType.Relu,
                                 bias=bias)
        else:
            nc.vector.tensor_scalar(out=dst, in0=src, scalar1=bias,
                                    scalar2=0.0, op0=ALU.add, op1=ALU.max)

    def copy(self, dst, src, width):
        nc = self.nc
        e = self._pick({k: v[width] for k, v in self.EVAC_COST.items()})
        if e == "act":
            nc.scalar.copy(dst, src)
        else:
            nc.vector.tensor_copy(dst, src)

    def agg(self, out, in0, in1):
        nc = self.nc
        e = self._pick(self.AGG_COST)
        if e == "dve":
            nc.vector.tensor_tensor(out=out, in0=in0, in1=in1,
                                    op=mybir.AluOpType.add)
        else:
            nc.gpsimd.tensor_tensor(out=out, in0=in0, in1=in1,
                                    op=mybir.AluOpType.add)


def _build_program(reps=1):
    pe_only = bool(int(os.environ.get("KERNEL_PE_ONLY", "0")))
    alu_only = bool(int(os.environ.get("KERNEL_ALU_ONLY", "0")))
    edge_full = os.environ.get("KERNEL_EDGE", "full") == "full"
    tplan = dict(TPLAN)
    if edge_full:
        for q in [1, 2, 4, 5, 7]:
            tplan[q] = ("edge", TPLAN[q][1])
    nc = bacc.Bacc(None)

    x2_d = nc.declare_dram_parameter("x2", [KA, 2, BC], F8, isOutput=False)
    g1_d = nc.declare_dram_parameter("g1", [KA, 2, 9 * 128], F8, isOutput=False)
    g2_d = nc.declare_dram_parameter("g2", [128, 9 * 128], BF16, isOutput=False)
    g2e_d = nc.declare_dram_parameter("g2e", [128, 128], BF16, isOutput=False)
    g3_d = nc.declare_dram_parameter("g3", [128, 9 * 64], BF16, isOutput=False)
    wp2t_d = nc.declare_dram_parameter("wp2t", [65, DOUT], BF16, isOutput=False)
    b1p_d = nc.declare_dram_parameter("b1p", [128, 1], F32, isOutput=False)
    b2p_d = nc.declare_dram_parameter("b2p", [128, 1], F32, isOutput=False)
    bzt_d = nc.declare_dram_parameter("bzt", [H, 1], F32, isOutput=False)
    # partition-major output: each partition writes contiguous 2KB per tile;
    # host permutes back to (BC, DOUT).
    out_d = nc.declare_dram_parameter("out", [128, BC // 128, DOUT], BF16,
                                      isOutput=True)
    out_r = out_d

    MM = mybir.MatmulPerfMode

    with tile.TileContext(nc) as tc:
        with (
            tc.tile_pool(name="const", bufs=1) as cp,
            tc.tile_pool(name="h1", bufs=2) as h1p,
            tc.tile_pool(name="tt", bufs=2) as tp,
            tc.tile_pool(name="h2", bufs=2) as h2p,
            tc.tile_pool(name="ot", bufs=3) as otp,
            tc.tile_pool(name="psm", bufs=3, space="PSUM") as psm,
            tc.tile_pool(name="psc", bufs=2, space="PSUM") as psc,
        ):
            x2_sb = cp.tile([KA, 2, BC], F8)
            g1_sb = cp.tile([KA, 2, 9 * 128], F8)
            g2_sb = cp.tile([128, 9 * 128], BF16)
            g2e_sb = cp.tile([128, 128], BF16)
            g3_sb = cp.tile([128, 9 * 64], BF16)
            wp2t_sb = cp.tile([65, DOUT], BF16)
            b1p_sb = cp.tile([128, 1], F32)
            b2p_sb = cp.tile([128, 1], F32)
            bzt_sb = cp.tile([H, 1], F32)
            z4_sb = cp.tile([65, 2 * TILE_N], BF16)   # manual double buffer

            nc.sync.dma_start(x2_sb[:, :, :], x2_d[:, :, :])
            nc.sync.dma_start(g1_sb[:, :, :], g1_d[:, :, :])
            nc.sync.dma_start(g2_sb[:], g2_d[:])
            nc.sync.dma_start(g2e_sb[:], g2e_d[:])
            nc.sync.dma_start(g3_sb[:], g3_d[:])
            nc.sync.dma_start(wp2t_sb[:], wp2t_d[:])
            nc.sync.dma_start(b1p_sb[:], b1p_d[:])
            nc.sync.dma_start(b2p_sb[:], b2p_d[:])
            nc.sync.dma_start(bzt_sb[:], bzt_d[:])
            nc.gpsimd.memset(z4_sb[64:65, :], 1.0)    # stage-D bias row

            sched = _Sched(nc)
            state = {}

            def a_pair(g, h1_sb, s0, nwin):
                """A matmuls for slots s0..s0+nwin-1 into one psum tile,
                one evac."""
                ps = psm.tile([128, 2 * TILE_N], F32, name="ps_m")
                if not alu_only:
                    for i in range(nwin):
                        s = s0 + i
                        nc.tensor.matmul(
                            ps[:, TILE_N * i:TILE_N * (i + 1)],
                            g1_sb[:, :, 128 * s:128 * (s + 1)],
                            x2_sb[:, :, TILE_N * g:TILE_N * (g + 1)],
                            start=True, stop=True,
                            perf_mode=MM.DoubleRow,
                        )
                w = nwin * TILE_N
                if not pe_only:
                    sched.evac(h1_sb[:, TILE_N * s0:TILE_N * s0 + w],
                               ps[:, 0:w], b1p_sb[:], w)

            def stage_a_pre(g):
                h1_sb = h1p.tile([128, 9 * TILE_N], BF16)
                state[("h1", g)] = h1_sb
                a_pair(g, h1_sb, 0, 2)        # w1, w4

            def stage_a_mid(g):
                h1_sb = state[("h1", g)]
                a_pair(g, h1_sb, 2, 2)        # w7, w0

            def stage_a_post(g):
                h1_sb = state[("h1", g)]
                a_pair(g, h1_sb, 4, 2)        # w2, w3
                a_pair(g, h1_sb, 6, 2)        # w5, w6
                a_pair(g, h1_sb, 8, 1)        # w8
                # aggregation: t windows (bf16 adds, DVE/GpSimd)
                t_sb = tp.tile([128, N_TSLOT * TILE_N], BF16)
                state[("t", g)] = t_sb

                def hw(w):
                    s = SLOT_OF[w]
                    return h1_sb[:, TILE_N * s:TILE_N * (s + 1)]

                def ts(q):
                    s = TSLOT[q]
                    return t_sb[:, TILE_N * s:TILE_N * (s + 1)]

                if not pe_only:
                    for q in [1, 2, 4, 5, 7]:
                        if tplan[q][0] != "sum":
                            continue
                        ws = tplan[q][1]
                        sched.agg(ts(q), hw(ws[0]), hw(ws[1]))

            def hw_of(h1_sb, w):
                sl = SLOT_OF[w]
                return h1_sb[:, TILE_N * sl:TILE_N * (sl + 1)]

            def b_rhs(q, h1_sb, t_sb):
                kind = tplan[q][0]
                if kind == "alias":
                    return hw_of(h1_sb, tplan[q][1])
                if kind == "alias_hi":
                    return hw_of(h1_sb, tplan[q][1])[64:128, :]
                sl = TSLOT[q]
                return t_sb[:, TILE_N * sl:TILE_N * (sl + 1)]

            def b_mm(q, h1_sb, t_sb, ps, off):
                kind = tplan[q][0]
                if kind == "edge":
                    ws = tplan[q][1]
                    for i, w in enumerate(ws):
                        lhsT = (g2e_sb[:, :] if w == 7
                                else g2_sb[:, 128 * q:128 * (q + 1)])
                        nc.tensor.matmul(
                            ps[:, off:off + TILE_N], lhsT,
                            hw_of(h1_sb, w),
                            start=(i == 0), stop=(i == len(ws) - 1),
                        )
                elif kind == "alias_hi":
                    nc.tensor.matmul(
                        ps[:, off:off + TILE_N],
                        g2_sb[64:128, 128 * q:128 * (q + 1)],
                        b_rhs(q, h1_sb, t_sb), start=True, stop=True,
                        tile_position=(64, 0),
                    )
                else:
                    nc.tensor.matmul(
                        ps[:, off:off + TILE_N],
                        g2_sb[:, 128 * q:128 * (q + 1)],
                        b_rhs(q, h1_sb, t_sb), start=True, stop=True,
                    )

            def stage_b(g):
                h1_sb = state.pop(("h1", g))
                t_sb = state.pop(("t", g))
                h2_sb = h2p.tile([128, 9 * TILE_N], BF16)
                state[("h2", g)] = h2_sb
                for q0 in (2, 4, 6, 0):
                    ps = psm.tile([128, 2 * TILE_N], F32, name="ps_m")
                    if not alu_only:
                        b_mm(q0, h1_sb, t_sb, ps, 0)
                        b_mm(q0 + 1, h1_sb, t_sb, ps, TILE_N)
                    if not pe_only:
                        sched.evac(h2_sb[:, TILE_N * q0:TILE_N * (q0 + 2)],
                                   ps[:, :], b2p_sb[:], 2 * TILE_N)
                ps = psm.tile([128, 2 * TILE_N], F32, name="ps_m")
                if not alu_only:
                    b_mm(8, h1_sb, t_sb, ps, 0)
                if not pe_only:
                    sched.evac(h2_sb[:, TILE_N * 8:TILE_N * 9],
                               ps[:, 0:TILE_N], b2p_sb[:], TILE_N)

            def stage_c(g):
                h2_sb = state.pop(("h2", g))
                ps_c = psc.tile([128, TILE_N], F32)
                order = (2, 3, 4, 5, 6, 7, 0, 1, 8)
                if not alu_only:
                    for i, p in enumerate(order):
                        nc.tensor.matmul(
                            ps_c[0:64, :],
                            g3_sb[:, 64 * p:64 * (p + 1)],
                            h2_sb[:, TILE_N * p:TILE_N * (p + 1)],
                            start=(i == 0), stop=(i == len(order) - 1),
                        )
                zoff = TILE_N * (g % 2)
                if not pe_only:
                    sched.evac(z4_sb[0:64, zoff:zoff + TILE_N], ps_c[0:64, :],
                               bzt_sb[:], TILE_N)

            def stage_d(g):
                zoff = TILE_N * (g % 2)
                ot_sb = otp.tile([128, 4 * DOUT], BF16)
                ps_d = psm.tile([128, 2 * TILE_N], F32, name="ps_m")
                if not alu_only:
                    for kk in range(4):
                        nc.tensor.matmul(
                            ps_d[:, DOUT * kk:DOUT * (kk + 1)],
                            z4_sb[0:65, zoff + 128 * kk:zoff + 128 * (kk + 1)],
                            wp2t_sb[:],
                            start=True, stop=True,
                        )
                if not pe_only:
                    sched.copy(ot_sb[:], ps_d[:], 2 * TILE_N)
                if not (pe_only or alu_only):
                    nc.sync.dma_start(out_r[:, 4 * g:4 * (g + 1), :], ot_sb[:])

            ntiles = int(os.environ.get("KERNEL_NTILES", str(NTILES)))

            def body():
                # deep pipeline: every stage consumes inputs >= 1 iteration
                # old, so DMA/agg chains never sit on the critical path.
                # stage C (psc pool, no psm WARs) is emitted between the A
                # pairs so the PE has independent work while A's psum tiles
                # wait on evacuations.
                for g in range(ntiles + 3):
                    if g < ntiles:
                        stage_a_pre(g)
                        stage_a_mid(g)
                    if 2 <= g <= ntiles + 1:
                        stage_c(g - 2)
                    if g < ntiles:
                        stage_a_post(g)
                    if 1 <= g <= ntiles:
                        stage_b(g - 1)
                    if 3 <= g:
                        stage_d(g - 3)

            if reps == 1:
                body()
            else:
                with tc.For_i(0, reps, 1):
                    body()

    nc.compile()
    return nc


_CACHE = {}


def kernel(**inputs):
    global LAST_RESULTS
    from concourse.bass_utils import run_bass_kernel_spmd

    x = np.ascontiguousarray(np.asarray(inputs["x"], np.float32))
    consts = _build_constants(
        inputs["A"], inputs["W1"], inputs["b1"], inputs["W2"], inputs["b2"],
        inputs["W3"], inputs["b3"], inputs["Wp1"], inputs["bp1"],
        inputs["Wp2"], inputs["bp2"],
    )

    reps = int(os.environ.get("BENCH_REPS", "1"))
    key = (reps,)
    if key not in _CACHE:
        _CACHE[key] = _build_program(reps=reps)
    nc = _CACHE[key]

    xf = x.reshape(B_TOTAL, NJ * DIN)
    in_maps = []
    for c in range(NCORES):
        m = dict(consts)
        m["x2"] = _pack_x(xf[c * BC:(c + 1) * BC])
        in_maps.append(m)

    res = run_bass_kernel_spmd(nc, in_maps, list(range(NCORES)))
    LAST_RESULTS = res
    outs = []
    for c in range(NCORES):
        o = np.asarray(res.results[c]["out"])        # (128, 64, 256)
        outs.append(np.ascontiguousarray(o.transpose(1, 0, 2))
                    .reshape(BC, DOUT))              # row c*128+p = o[p, c]
    return np.concatenate(outs, axis=0).astype(np.float32)


if __name__ == "__main__":
    import reference
    ins = {k: np.asarray(v) for k, v in reference.setup_inputs().items()}
    exp = np.asarray(reference.reference(**reference.setup_inputs()))
    os.environ.setdefault("BENCH_REPS", "1")
    act = kernel(**ins)
    scale = float(np.abs(exp).max())
    err = float(np.abs(act - exp).max())
    print(f"scale={scale:.4f} abserr={err:.3e} relerr={err / scale:.3e}")
